# revision 1
# baseline (speedup 1.0000x reference)
"""Trainium2 Bass kernel for a GroupNorm + cross-attention block.

Reference computation (shapes hardcoded):
  x:[2,512,16,16,16] -> GroupNorm(32 groups over (16ch x 4096 spatial))
  q = xn_seq @ Wq ; k,v = context @ Wk/Wv  (context:[2,1024,768])
  attn = softmax(q k^T / 8) ; out = (attn v) @ Wo + bo + residual
  output: [2,512,16,16,16]

Sharding: 8 cores = 2 batches x 4 sequence-quarters (1024 of 4096 voxel
tokens each). Each core computes all 8 heads for its token slice; the only
cross-core communication is a [8,8] f32 AllReduce of GroupNorm statistics
within each 4-core batch group.

Device layout notes:
 - Everything keeps channels/inner-dim on the partition axis, so no
   transposes are needed anywhere on device: the host feeds context^T.
 - GroupNorm is folded into the q projection: Wq rows are scaled by the
   per-channel A = gamma*rstd and a rank-1 offset q0 = Wq^T B is added,
   so normalized x is never materialized.
 - Softmax runs without max-subtraction (scores are O(+-6) here, safe in
   fp32) with exp on the scalar engine, reading score PSUM directly.
 - The softmax denominator comes from an extra all-ones column appended to
   the V tile (lhsT [128,65]) so row 64 of the AV matmul accumulates
   sum(exp) for free.
 - Matmuls use float32r (full PE speed at >=256 free dim, ~fp32 accuracy).
"""

import os
from contextlib import ExitStack, nullcontext

import numpy as np

import concourse.bass as bass
import concourse.mybir as mybir
import concourse.tile as tile
from concourse import bacc, bass_utils

F32 = mybir.dt.float32
F32R = mybir.dt.float32r
BF16 = mybir.dt.bfloat16
AF = mybir.ActivationFunctionType
ALU = mybir.AluOpType

B = 2
C = 512
N = 4096            # voxel tokens per batch (16*16*16)
NL = 1024           # tokens per core (N / 4)
CTX = 1024
CTXD = 768
HEADS = 8
HD = 64
INNER = HEADS * HD  # 512
GROUPS = 32
EPS = 1e-5
SCALE = HD ** -0.5

CT = C // 128       # 4 channel tiles
KT = CTXD // 128    # 6 context-dim tiles
MT = CTX // 128     # 8 ctx row tiles
PAIRS = HEADS // 2  # 4 head-pair tiles (128 rows each)

_CACHED_NC = None


def build_nc(loop_iters=1, skip_collective=False):
    # Bacc (not raw Bass): its finalize() runs the wait-splitting passes
    # (move_matmul_waits_to_ldweights / generate_event_semaphores) that the
    # TRN2 ISA requires — walrus rejects multi-wait matmuls otherwise.
    # loop_iters > 1 wraps the body in a device-side For_i so per-iteration
    # device time can be measured without per-dispatch overhead.
    nc = bacc.Bacc("TRN2", target_bir_lowering=False, debug=False, num_devices=8)

    x_d = nc.dram_tensor("x_l", [CT, 128, NL], F32R, kind="ExternalInput")
    ctxT_d = nc.dram_tensor("ctxT", [KT, 128, CTX], F32R, kind="ExternalInput")
    wq_d = nc.dram_tensor("wq", [CT, 128, INNER], F32R, kind="ExternalInput")
    wk_d = nc.dram_tensor("wk", [KT, 128, INNER], F32R, kind="ExternalInput")
    wv_d = nc.dram_tensor("wv", [KT, 128, INNER], F32R, kind="ExternalInput")
    wo_d = nc.dram_tensor("wo", [CT, 128, C], F32R, kind="ExternalInput")
    gamma_d = nc.dram_tensor("gamma_t", [128, CT], F32, kind="ExternalInput")
    beta_d = nc.dram_tensor("beta_t", [128, CT], F32, kind="ExternalInput")
    bo_d = nc.dram_tensor("bo_t", [128, CT], F32, kind="ExternalInput")
    self_fwd_d = nc.dram_tensor("sel_fwd", [128, 8], F32, kind="ExternalInput")
    sel_bwd_d = nc.dram_tensor("sel_bwd", [8, 128], F32, kind="ExternalInput")
    out_d = nc.dram_tensor("out_l", [CT, 128, NL], F32, kind="ExternalOutput")

    stats_in_d = nc.dram_tensor("stats_in", [8, 8], F32)
    stats_out_d = nc.dram_tensor("stats_out", [8, 8], F32)
    # scratch for partition-broadcasting softmax denominators (DRAM bounce:
    # SBUF/PSUM sources cannot be read with partition-stride 0, DRAM can)
    den_d = nc.dram_tensor("den_scratch", [HEADS, 2, 512], F32)

    with tile.TileContext(nc) as tc, ExitStack() as ctx:
        consts = ctx.enter_context(tc.tile_pool(name="consts", bufs=1))
        wpool = ctx.enter_context(tc.tile_pool(name="weights", bufs=1))
        xpool = ctx.enter_context(tc.tile_pool(name="x", bufs=1))
        cxpool = ctx.enter_context(tc.tile_pool(name="ctx", bufs=1))
        ktpool = ctx.enter_context(tc.tile_pool(name="kt", bufs=1))
        vpool = ctx.enter_context(tc.tile_pool(name="v", bufs=1))
        qpool = ctx.enter_context(tc.tile_pool(name="qt", bufs=1))
        epool = ctx.enter_context(tc.tile_pool(name="e", bufs=12))
        otpool = ctx.enter_context(tc.tile_pool(name="ot", bufs=1))
        spool = ctx.enter_context(tc.tile_pool(name="small", bufs=4))
        dpool = ctx.enter_context(tc.tile_pool(name="den", bufs=4))
        opool = ctx.enter_context(tc.tile_pool(name="outs", bufs=3))

        pp = ctx.enter_context(tc.tile_pool(name="pproj", bufs=2, space="PSUM"))
        pst = ctx.enter_context(tc.tile_pool(name="pst", bufs=2, space="PSUM"))
        pav = ctx.enter_context(tc.tile_pool(name="pav", bufs=2, space="PSUM"))

        loop_cm = tc.For_i(0, loop_iters, 1) if loop_iters > 1 else nullcontext()
        with loop_cm:

            # ---- constant/weight loads ----
            sel_fwd = consts.tile([128, 8], F32, tag="sel_fwd")
            nc.sync.dma_start(out=sel_fwd, in_=self_fwd_d[:, :])
            sel_bwd = consts.tile([8, 128], F32, tag="sel_bwd")
            nc.sync.dma_start(out=sel_bwd, in_=sel_bwd_d[:, :])
            gamma_sb = consts.tile([128, CT], F32, tag="gamma")
            nc.sync.dma_start(out=gamma_sb, in_=gamma_d[:, :])
            beta_sb = consts.tile([128, CT], F32, tag="beta")
            nc.sync.dma_start(out=beta_sb, in_=beta_d[:, :])
            bo_sb = consts.tile([128, CT], F32, tag="bo")
            nc.sync.dma_start(out=bo_sb, in_=bo_d[:, :])
            eps_sb = consts.tile([128, 1], F32, tag="eps")
            nc.vector.memset(eps_sb, EPS)
            ones8_sb = consts.tile([128, 8], F32, tag="ones8sb")
            nc.vector.memset(ones8_sb, 1.0)

            x_sb = []
            for t in range(CT):
                xt = xpool.tile([128, NL], F32R, tag=f"x{t}")
                nc.sync.dma_start(out=xt, in_=x_d[t])
                x_sb.append(xt)
            ctx_sb = []
            for kk in range(KT):
                ct_ = cxpool.tile([128, CTX], F32R, tag=f"ctx{kk}")
                nc.sync.dma_start(out=ct_, in_=ctxT_d[kk])
                ctx_sb.append(ct_)
            wq_sb, wk_sb, wv_sb, wo_sb = [], [], [], []
            for t in range(CT):
                w = wpool.tile([128, INNER], F32R, tag=f"wq{t}")
                nc.sync.dma_start(out=w, in_=wq_d[t])
                wq_sb.append(w)
            for kk in range(KT):
                w = wpool.tile([128, INNER], F32R, tag=f"wk{kk}")
                nc.sync.dma_start(out=w, in_=wk_d[kk])
                wk_sb.append(w)
            for kk in range(KT):
                w = wpool.tile([128, INNER], F32R, tag=f"wv{kk}")
                nc.sync.dma_start(out=w, in_=wv_d[kk])
                wv_sb.append(w)
            for t in range(CT):
                w = wpool.tile([128, C], F32R, tag=f"wo{t}")
                nc.sync.dma_start(out=w, in_=wo_d[t])
                wo_sb.append(w)

            # ---- GroupNorm statistics ----
            # per-channel (mean, E[x^2]) over the local token slice, group-reduced
            # on the PE with sel_fwd (value 1/64: 16 channels x 4 cores), then
            # AllReduced within the batch group.
            ps_stats = pp.tile([128, 512], F32, tag="proj")
            for t in range(CT):
                st6 = spool.tile([128, 2, 6], F32, tag="bn6")
                for sg in range(2):
                    nc.vector.bn_stats(
                        out=st6[:, sg, :], in_=x_sb[t][:, sg * 512:(sg + 1) * 512]
                    )
                mv = spool.tile([128, 2], F32, tag="mv")
                nc.vector.bn_aggr(out=mv, in_=st6)
                s12 = spool.tile([128, 2], F32, tag="s12")
                nc.vector.tensor_copy(out=s12[:, 0:1], in_=mv[:, 0:1])
                nc.vector.tensor_mul(s12[:, 1:2], mv[:, 0:1], mv[:, 0:1])
                nc.vector.tensor_add(s12[:, 1:2], s12[:, 1:2], mv[:, 1:2])
                nc.tensor.matmul(
                    ps_stats[0:8, t * 2:t * 2 + 2], lhsT=sel_fwd, rhs=s12,
                    start=True, stop=True,
                )
            stats_sb = spool.tile([8, 8], F32, tag="gst")
            nc.vector.tensor_copy(out=stats_sb, in_=ps_stats[0:8, 0:8])
            nc.sync.dma_start(out=stats_in_d[:, :], in_=stats_sb)
            if skip_collective:
                # timing-only variant: collectives inside a device-side For_i
                # desync the mesh on the 2nd iteration, so the timing loop
                # substitutes a local DRAM copy (output values are wrong by a
                # constant stats factor; latency profile is comparable).
                nc.sync.dma_start(out=stats_out_d[:, :], in_=stats_in_d[:, :])
            else:
                nc.gpsimd.collective_compute(
                    "AllReduce",
                    ALU.add,
                    replica_groups=[[0, 1, 2, 3], [4, 5, 6, 7]],
                    ins=[stats_in_d[:, :]],
                    outs=[stats_out_d[:, :]],
                )
            g_sb = spool.tile([8, 8], F32, tag="gout")
            nc.sync.dma_start(out=g_sb, in_=stats_out_d[:, :])

            # Per channel-tile affine coefficients:
            #   A = gamma * rstd,  B = beta - mean*rstd*gamma
            # rstd = exp(-0.5 * ln(var + eps)) keeps everything in the single
            # natural_log_exp activation table set shared with the softmax exp.
            cA, cB = [], []
            for t in range(CT):
                m1 = g_sb[:, 2 * t:2 * t + 1]
                m2 = g_sb[:, 2 * t + 1:2 * t + 2]
                var8 = spool.tile([8, 1], F32, tag=f"var{t}")
                nc.vector.tensor_mul(var8, m1, m1)
                nc.vector.tensor_sub(var8, m2, var8)
                nc.scalar.activation(var8, var8, AF.Ln, bias=eps_sb[0:8, :])
                nc.scalar.activation(var8, var8, AF.Exp, scale=-0.5)  # rstd
                ab8 = spool.tile([8, 2], F32, tag=f"ab{t}")
                nc.vector.tensor_copy(out=ab8[:, 0:1], in_=var8)
                nc.vector.tensor_mul(ab8[:, 1:2], m1, var8)  # mean*rstd
                ps_ab = pp.tile([128, 512], F32, tag="proj")
                nc.tensor.matmul(
                    ps_ab[:, 0:2], lhsT=sel_bwd, rhs=ab8, start=True, stop=True
                )
                a_t = spool.tile([128, 1], F32, tag=f"cA{t}")
                b_t = spool.tile([128, 1], F32, tag=f"cB{t}")
                nc.vector.tensor_mul(a_t, ps_ab[:, 0:1], gamma_sb[:, t:t + 1])
                nc.vector.tensor_mul(b_t, ps_ab[:, 1:2], gamma_sb[:, t:t + 1])
                nc.vector.tensor_sub(b_t, beta_sb[:, t:t + 1], b_t)
                cA.append(a_t)
                cB.append(b_t)

            # q0 = Wq^T B per head-pair tile (before Wq is scaled in place).
            q0_sb = []
            for j in range(PAIRS):
                pq0 = pp.tile([128, 512], F32, tag="proj")
                for t in range(CT):
                    nc.tensor.matmul(
                        pq0[:, 0:1],
                        lhsT=wq_sb[t][:, j * 128:(j + 1) * 128].bitcast(F32),
                        rhs=cB[t],
                        start=(t == 0), stop=(t == CT - 1),
                    )
                q0 = spool.tile([128, 1], F32, tag=f"q0{j}")
                nc.vector.tensor_copy(out=q0, in_=pq0[:, 0:1])
                q0_sb.append(q0)
            # Fold A into Wq rows in place (q0 above already consumed raw Wq).
            for t in range(CT):
                nc.vector.tensor_scalar_mul(
                    out=wq_sb[t], in0=wq_sb[t], scalar1=cA[t]
                )

            # ---- V = context @ Wv  (per ctx row-tile, all heads; +ones col) ----
            v_sb = []
            for m in range(MT):
                pv = pp.tile([128, 512], F32, tag="proj")
                for kk in range(KT):
                    nc.tensor.matmul(
                        pv,
                        lhsT=(ctx_sb[kk][:, m * 128:(m + 1) * 128]),
                        rhs=(wv_sb[kk]),
                        start=(kk == 0), stop=(kk == KT - 1),
                    )
                vt = vpool.tile([128, HEADS, HD + 1], BF16, tag=f"v{m}")
                for h in range(HEADS):
                    nc.vector.tensor_copy(
                        out=vt[:, h, 0:HD], in_=pv[:, h * HD:(h + 1) * HD]
                    )
                nc.vector.tensor_copy(
                    out=vt[:, :, HD:HD + 1],
                    in_=ones8_sb.rearrange("p (f one) -> p f one", one=1),
                )
                v_sb.append(vt)

            # ---- K^T = (context @ Wk)^T  (per head-pair tile) ----
            kT_sb = []
            for j in range(PAIRS):
                kt_ = ktpool.tile([128, CTX], BF16, tag=f"kT{j}")
                for cc in range(2):
                    pk = pp.tile([128, 512], F32, tag="proj")
                    for kk in range(KT):
                        nc.tensor.matmul(
                            pk,
                            lhsT=(wk_sb[kk][:, j * 128:(j + 1) * 128]),
                            rhs=(ctx_sb[kk][:, cc * 512:(cc + 1) * 512]),
                            start=(kk == 0), stop=(kk == KT - 1),
                        )
                    nc.vector.tensor_copy(
                        out=kt_[:, cc * 512:(cc + 1) * 512], in_=pk
                    )
                kT_sb.append(kt_)

            # ---- q^T (per head-pair tile), GroupNorm pre-folded ----
            qT_sb = []
            for j in range(PAIRS):
                qt_ = qpool.tile([128, NL], BF16, tag=f"qT{j}")
                for ncc in range(2):
                    pq = pp.tile([128, 512], F32, tag="proj")
                    for t in range(CT):
                        nc.tensor.matmul(
                            pq,
                            lhsT=(wq_sb[t][:, j * 128:(j + 1) * 128]),
                            rhs=(x_sb[t][:, ncc * 512:(ncc + 1) * 512]),
                            start=(t == 0), stop=(t == CT - 1),
                        )
                    nc.vector.tensor_scalar_add(
                        out=qt_[:, ncc * 512:(ncc + 1) * 512],
                        in0=pq,
                        scalar1=q0_sb[j],
                    )
                qT_sb.append(qt_)

            # ---- attention per (head-pair, token-chunk of 512) ----
            # Scores for both heads of a pair are issued back-to-back with
            # tile_position (0,0)/(64,0): K=64 matmuls on distinct PE array
            # row-groups run concurrently (~2x). One exp covers both heads.
            ot_sb = [
                otpool.tile([128, NL], F32R, tag=f"ot{j}", name=f"ot{j}")
                for j in range(PAIRS)
            ]
            for ncc in range(2):
                for j in range(PAIRS):
                    cs = slice(ncc * 512, (ncc + 1) * 512)
                    e_tiles = []
                    for m in range(MT):
                        stp = pst.tile([128, 1024], F32, tag="st")
                        nc.tensor.matmul(
                            stp[:, 0:512],
                            lhsT=kT_sb[j][0:HD, m * 128:(m + 1) * 128],
                            rhs=qT_sb[j][0:HD, cs],
                            start=True, stop=True,
                        )
                        nc.tensor.matmul(
                            stp[:, 512:1024],
                            lhsT=kT_sb[j][HD:128, m * 128:(m + 1) * 128],
                            rhs=qT_sb[j][HD:128, cs],
                            start=True, stop=True,
                        )
                        et = epool.tile([128, 1024], BF16, tag="e")
                        nc.scalar.activation(et, stp, AF.Exp, scale=SCALE)
                        e_tiles.append(et)
                    for half in range(2):
                        h = 2 * j + half
                        rs = slice(half * HD, (half + 1) * HD)
                        es = slice(half * 512, (half + 1) * 512)
                        avp = pav.tile([HD + 1, 512], F32, tag="av")
                        for m in range(MT):
                            nc.tensor.matmul(
                                avp,
                                lhsT=v_sb[m][:, h, :],
                                rhs=e_tiles[m][:, es],
                                start=(m == 0), stop=(m == MT - 1),
                            )
                        rec1 = dpool.tile([1, 512], F32, tag="rec1")
                        nc.vector.reciprocal(out=rec1, in_=avp[HD:HD + 1, :])
                        nc.sync.dma_start(out=den_d[h, ncc, :], in_=rec1)
                        den_row = den_d[h, ncc, :]
                        den_bc_ap = bass.AP(
                            tensor=den_row.tensor,
                            offset=den_row.offset,
                            ap=[[0, HD], [1, 512]],
                        )
                        rec = dpool.tile([HD, 512], F32, tag="rec")
                        nc.sync.dma_start(out=rec, in_=den_bc_ap)
                        nc.vector.tensor_mul(ot_sb[j][rs, cs], avp[0:HD, :], rec)

                cs = slice(ncc * 512, (ncc + 1) * 512)
                # out = OT^T Wo + bo + residual for this token chunk; overlaps
                # with the next chunk's attention on other engines.
                for t in range(CT):
                    po = pp.tile([128, 512], F32, tag="proj")
                    for jj in range(PAIRS):
                        nc.tensor.matmul(
                            po,
                            lhsT=(wo_sb[jj][:, t * 128:(t + 1) * 128]),
                            rhs=(ot_sb[jj][:, cs]),
                            start=(jj == 0), stop=(jj == PAIRS - 1),
                        )
                    res = opool.tile([128, 512], F32, tag="res")
                    nc.vector.scalar_tensor_tensor(
                        out=res,
                        in0=po,
                        scalar=bo_sb[:, t:t + 1],
                        in1=x_sb[t][:, cs],
                        op0=ALU.add,
                        op1=ALU.add,
                    )
                    nc.sync.dma_start(out=out_d[t, :, cs], in_=res)

    nc.finalize()
    return nc


def _host_prep(x, context, gamma, beta, Wq, Wk, Wv, Wo, bo):
    """Build the 8 per-core input maps (host-side slicing/transposes only)."""
    x2 = np.ascontiguousarray(x, np.float32).reshape(B, C, N)
    ctx = np.ascontiguousarray(context, np.float32)

    sel_fwd = np.zeros((128, 8), np.float32)
    for p in range(128):
        sel_fwd[p, p // 16] = 1.0 / 64.0  # 16 channels x 4 cores
    sel_bwd = np.zeros((8, 128), np.float32)
    for p in range(128):
        sel_bwd[p // 16, p] = 1.0

    shared = {
        "wq": np.ascontiguousarray(Wq, np.float32).reshape(CT, 128, INNER),
        "wk": np.ascontiguousarray(Wk, np.float32).reshape(KT, 128, INNER),
        "wv": np.ascontiguousarray(Wv, np.float32).reshape(KT, 128, INNER),
        "wo": np.ascontiguousarray(Wo, np.float32).reshape(CT, 128, C),
        "gamma_t": np.ascontiguousarray(
            np.asarray(gamma, np.float32).reshape(CT, 128).T
        ),
        "beta_t": np.ascontiguousarray(
            np.asarray(beta, np.float32).reshape(CT, 128).T
        ),
        "bo_t": np.ascontiguousarray(np.asarray(bo, np.float32).reshape(CT, 128).T),
        "sel_fwd": sel_fwd,
        "sel_bwd": sel_bwd,
        "stats_in": np.zeros((8, 8), np.float32),
        "stats_out": np.zeros((8, 8), np.float32),
    }

    in_maps = []
    for core in range(8):
        b, qt = core // 4, core % 4
        m = dict(shared)
        m["x_l"] = np.ascontiguousarray(
            x2[b, :, qt * NL:(qt + 1) * NL]
        ).reshape(CT, 128, NL)
        m["ctxT"] = np.ascontiguousarray(ctx[b].T).reshape(KT, 128, CTX)
        in_maps.append(m)
    return in_maps


def _assemble(results):
    out = np.zeros((B, C, N), np.float32)
    for core in range(8):
        b, qt = core // 4, core % 4
        out[b, :, qt * NL:(qt + 1) * NL] = results[core]["out_l"].reshape(C, NL)
    return out.reshape(B, C, 16, 16, 16)


def run(inputs, trace=False):
    global _CACHED_NC
    if _CACHED_NC is None:
        _CACHED_NC = build_nc()
    nc = _CACHED_NC
    in_maps = _host_prep(**inputs)
    # stats_in/stats_out are internal dram tensors, not ExternalInputs
    for m in in_maps:
        m.pop("stats_in")
        m.pop("stats_out")
    bkr = bass_utils.run_bass_kernel_spmd(
        nc, in_maps, list(range(8)), trace=trace
    )
    return _assemble(bkr.results), bkr


def kernel(**inputs):
    out, _ = run(inputs)
    return out



# revision 2
# speedup vs baseline: 17.5855x; 17.5855x over previous
"""Trainium2 Bass kernel for a GroupNorm + cross-attention block.

Reference computation (shapes hardcoded):
  x:[2,512,16,16,16] -> GroupNorm(32 groups over (16ch x 4096 spatial))
  q = xn_seq @ Wq ; k,v = context @ Wk/Wv  (context:[2,1024,768])
  attn = softmax(q k^T / 8) ; out = (attn v) @ Wo + bo + residual
  output: [2,512,16,16,16]

Sharding: 8 cores = 2 batches x 4 sequence-quarters (1024 of 4096 voxel
tokens each). Each core computes all 8 heads for its token slice; the only
cross-core communication is a [8,8] f32 AllReduce of GroupNorm statistics
within each 4-core batch group.

Device layout notes:
 - Everything keeps channels/inner-dim on the partition axis, so no
   transposes are needed anywhere on device: the host feeds context^T.
 - GroupNorm is folded into the q projection: Wq rows are scaled by the
   per-channel A = gamma*rstd and a rank-1 offset q0 = Wq^T B is added,
   so normalized x is never materialized.
 - Softmax runs without max-subtraction (scores are O(+-6) here, safe in
   fp32) with exp on the scalar engine, reading score PSUM directly.
 - The softmax denominator comes from an extra all-ones column appended to
   the V tile (lhsT [128,65]) so row 64 of the AV matmul accumulates
   sum(exp) for free.
 - Matmuls use float32r (full PE speed at >=256 free dim, ~fp32 accuracy).

Pipeline structure (the scheduling-critical part):
 - Input DMAs are ordered x, ctx, wk, wq, wv, wo so the GroupNorm stats
   chain and the first head-pair's K^T/q^T projections can start as early
   as possible.
 - The K^T/q^T projections are interleaved PER HEAD-PAIR with that pair's
   attention (scores -> exp -> AV), instead of projecting everything first:
   the scalar engine's 64us of exp work starts ~25us into the kernel and
   overlaps all remaining tensor work.
 - PSUM->SBUF evacuations in the projection path (V tiles, K^T tiles, q^T
   bias-add) run on the scalar engine (Copy/Identity are in the same
   activation table set as Exp/Ln -- no table switch), keeping the vector
   engine free for reciprocals/normalization.
"""

import os
from contextlib import ExitStack, nullcontext

import numpy as np

import concourse.bass as bass
import concourse.mybir as mybir
import concourse.tile as tile
from concourse import bacc, bass_utils

F32 = mybir.dt.float32
F32R = mybir.dt.float32r
BF16 = mybir.dt.bfloat16
AF = mybir.ActivationFunctionType
ALU = mybir.AluOpType

B = 2
C = 512
N = 4096            # voxel tokens per batch (16*16*16)
NL = 1024           # tokens per core (N / 4)
CTX = 1024
CTXD = 768
HEADS = 8
HD = 64
INNER = HEADS * HD  # 512
GROUPS = 32
EPS = 1e-5
SCALE = HD ** -0.5

CT = C // 128       # 4 channel tiles
KT = CTXD // 128    # 6 context-dim tiles
MT = CTX // 128     # 8 ctx row tiles
PAIRS = HEADS // 2  # 4 head-pair tiles (128 rows each)

_CACHED_NC = None


def build_nc(loop_iters=1, skip_collective=False):
    # Bacc (not raw Bass): its finalize() runs the wait-splitting passes
    # (move_matmul_waits_to_ldweights / generate_event_semaphores) that the
    # TRN2 ISA requires — walrus rejects multi-wait matmuls otherwise.
    # loop_iters > 1 wraps the body in a device-side For_i so per-iteration
    # device time can be measured without per-dispatch overhead.
    nc = bacc.Bacc("TRN2", target_bir_lowering=False, debug=False, num_devices=8)

    x_d = nc.dram_tensor("x_l", [CT, 128, NL], F32R, kind="ExternalInput")
    ctxT_d = nc.dram_tensor("ctxT", [KT, 128, CTX], F32R, kind="ExternalInput")
    wq_d = nc.dram_tensor("wq", [CT, 128, INNER], F32R, kind="ExternalInput")
    wk_d = nc.dram_tensor("wk", [KT, 128, INNER], F32R, kind="ExternalInput")
    wv_d = nc.dram_tensor("wv", [KT, 128, INNER], F32R, kind="ExternalInput")
    wo_d = nc.dram_tensor("wo", [CT, 128, C], F32R, kind="ExternalInput")
    gamma_d = nc.dram_tensor("gamma_t", [128, CT], F32, kind="ExternalInput")
    beta_d = nc.dram_tensor("beta_t", [128, CT], F32, kind="ExternalInput")
    bo_d = nc.dram_tensor("bo_t", [128, CT], F32, kind="ExternalInput")
    self_fwd_d = nc.dram_tensor("sel_fwd", [128, 8], F32, kind="ExternalInput")
    sel_bwd_d = nc.dram_tensor("sel_bwd", [8, 128], F32, kind="ExternalInput")
    out_d = nc.dram_tensor("out_l", [CT, 128, NL], F32, kind="ExternalOutput")

    stats_in_d = nc.dram_tensor("stats_in", [8, 8], F32)
    stats_out_d = nc.dram_tensor("stats_out", [8, 8], F32)
    # scratch for partition-broadcasting softmax denominators (DRAM bounce:
    # SBUF/PSUM sources cannot be read with partition-stride 0, DRAM can)
    den_d = nc.dram_tensor("den_scratch", [HEADS, 2, 512], F32)

    with tile.TileContext(nc) as tc, ExitStack() as ctx:
        consts = ctx.enter_context(tc.tile_pool(name="consts", bufs=1))
        wpool = ctx.enter_context(tc.tile_pool(name="weights", bufs=1))
        xpool = ctx.enter_context(tc.tile_pool(name="x", bufs=1))
        cxpool = ctx.enter_context(tc.tile_pool(name="ctx", bufs=1))
        ktpool = ctx.enter_context(tc.tile_pool(name="kt", bufs=1))
        vpool = ctx.enter_context(tc.tile_pool(name="v", bufs=1))
        qpool = ctx.enter_context(tc.tile_pool(name="qt", bufs=1))
        epool = ctx.enter_context(tc.tile_pool(name="e", bufs=12))
        otpool = ctx.enter_context(tc.tile_pool(name="ot", bufs=1))
        spool = ctx.enter_context(tc.tile_pool(name="small", bufs=4))
        dpool = ctx.enter_context(tc.tile_pool(name="den", bufs=4))
        opool = ctx.enter_context(tc.tile_pool(name="outs", bufs=3))

        pp = ctx.enter_context(tc.tile_pool(name="pproj", bufs=2, space="PSUM"))
        pst = ctx.enter_context(tc.tile_pool(name="pst", bufs=2, space="PSUM"))
        pav = ctx.enter_context(tc.tile_pool(name="pav", bufs=2, space="PSUM"))

        loop_cm = tc.For_i(0, loop_iters, 1) if loop_iters > 1 else nullcontext()
        with loop_cm:

            # ---- constant loads ----
            sel_fwd = consts.tile([128, 8], F32, tag="sel_fwd")
            nc.sync.dma_start(out=sel_fwd, in_=self_fwd_d[:, :])
            sel_bwd = consts.tile([8, 128], F32, tag="sel_bwd")
            nc.sync.dma_start(out=sel_bwd, in_=sel_bwd_d[:, :])
            gamma_sb = consts.tile([128, CT], F32, tag="gamma")
            nc.sync.dma_start(out=gamma_sb, in_=gamma_d[:, :])
            beta_sb = consts.tile([128, CT], F32, tag="beta")
            nc.sync.dma_start(out=beta_sb, in_=beta_d[:, :])
            bo_sb = consts.tile([128, CT], F32, tag="bo")
            nc.sync.dma_start(out=bo_sb, in_=bo_d[:, :])
            eps_sb = consts.tile([128, 1], F32, tag="eps")
            nc.vector.memset(eps_sb, EPS)
            ones8_sb = consts.tile([128, 8], F32, tag="ones8sb")
            nc.vector.memset(ones8_sb, 1.0)

            # ---- input loads, ordered by first use ----
            # x first (stats chain is the longest dependency chain), then ctx
            # + wk + wq (first head-pair's projections), wv, wo last.
            x_sb = []
            for t in range(CT):
                xt = xpool.tile([128, NL], F32R, tag=f"x{t}")
                nc.sync.dma_start(out=xt, in_=x_d[t])
                x_sb.append(xt)
            ctx_sb = []
            for kk in range(KT):
                ct_ = cxpool.tile([128, CTX], F32R, tag=f"ctx{kk}")
                nc.sync.dma_start(out=ct_, in_=ctxT_d[kk])
                ctx_sb.append(ct_)
            wk_sb = []
            for kk in range(KT):
                w = wpool.tile([128, INNER], F32R, tag=f"wk{kk}")
                nc.sync.dma_start(out=w, in_=wk_d[kk])
                wk_sb.append(w)
            wq_sb = []
            for t in range(CT):
                w = wpool.tile([128, INNER], F32R, tag=f"wq{t}")
                nc.sync.dma_start(out=w, in_=wq_d[t])
                wq_sb.append(w)
            wv_sb = []
            for kk in range(KT):
                w = wpool.tile([128, INNER], F32R, tag=f"wv{kk}")
                nc.sync.dma_start(out=w, in_=wv_d[kk])
                wv_sb.append(w)
            wo_sb = []
            for t in range(CT):
                w = wpool.tile([128, C], F32R, tag=f"wo{t}")
                nc.sync.dma_start(out=w, in_=wo_d[t])
                wo_sb.append(w)

            # ---- GroupNorm statistics ----
            # per-channel (mean, E[x^2]) over the local token slice, group-reduced
            # on the PE with sel_fwd (value 1/64: 16 channels x 4 cores), then
            # AllReduced within the batch group.
            ps_stats = pp.tile([128, 512], F32, tag="proj")
            for t in range(CT):
                st6 = spool.tile([128, 2, 6], F32, tag="bn6")
                for sg in range(2):
                    nc.vector.bn_stats(
                        out=st6[:, sg, :], in_=x_sb[t][:, sg * 512:(sg + 1) * 512]
                    )
                mv = spool.tile([128, 2], F32, tag="mv")
                nc.vector.bn_aggr(out=mv, in_=st6)
                s12 = spool.tile([128, 2], F32, tag="s12")
                nc.vector.tensor_copy(out=s12[:, 0:1], in_=mv[:, 0:1])
                nc.vector.tensor_mul(s12[:, 1:2], mv[:, 0:1], mv[:, 0:1])
                nc.vector.tensor_add(s12[:, 1:2], s12[:, 1:2], mv[:, 1:2])
                nc.tensor.matmul(
                    ps_stats[0:8, t * 2:t * 2 + 2], lhsT=sel_fwd, rhs=s12,
                    start=True, stop=True,
                )
            stats_sb = spool.tile([8, 8], F32, tag="gst")
            nc.vector.tensor_copy(out=stats_sb, in_=ps_stats[0:8, 0:8])
            nc.sync.dma_start(out=stats_in_d[:, :], in_=stats_sb)
            if skip_collective:
                # timing-only variant: collectives inside a device-side For_i
                # desync the mesh on the 2nd iteration, so the timing loop
                # substitutes a local DRAM copy (output values are wrong by a
                # constant stats factor; latency profile is comparable).
                nc.sync.dma_start(out=stats_out_d[:, :], in_=stats_in_d[:, :])
            else:
                nc.gpsimd.collective_compute(
                    "AllReduce",
                    ALU.add,
                    replica_groups=[[0, 1, 2, 3], [4, 5, 6, 7]],
                    ins=[stats_in_d[:, :]],
                    outs=[stats_out_d[:, :]],
                )
            g_sb = spool.tile([8, 8], F32, tag="gout")
            nc.sync.dma_start(out=g_sb, in_=stats_out_d[:, :])

            # Per channel-tile affine coefficients:
            #   A = gamma * rstd,  B = beta - mean*rstd*gamma
            # rstd = exp(-0.5 * ln(var + eps)) keeps everything in the single
            # natural_log_exp activation table set shared with the softmax exp.
            cA, cB = [], []
            for t in range(CT):
                m1 = g_sb[:, 2 * t:2 * t + 1]
                m2 = g_sb[:, 2 * t + 1:2 * t + 2]
                var8 = spool.tile([8, 1], F32, tag=f"var{t}")
                nc.vector.tensor_mul(var8, m1, m1)
                nc.vector.tensor_sub(var8, m2, var8)
                nc.scalar.activation(var8, var8, AF.Ln, bias=eps_sb[0:8, :])
                nc.scalar.activation(var8, var8, AF.Exp, scale=-0.5)  # rstd
                ab8 = spool.tile([8, 2], F32, tag=f"ab{t}")
                nc.vector.tensor_copy(out=ab8[:, 0:1], in_=var8)
                nc.vector.tensor_mul(ab8[:, 1:2], m1, var8)  # mean*rstd
                ps_ab = pp.tile([128, 512], F32, tag="proj")
                nc.tensor.matmul(
                    ps_ab[:, 0:2], lhsT=sel_bwd, rhs=ab8, start=True, stop=True
                )
                a_t = spool.tile([128, 1], F32, tag=f"cA{t}")
                b_t = spool.tile([128, 1], F32, tag=f"cB{t}")
                nc.vector.tensor_mul(a_t, ps_ab[:, 0:1], gamma_sb[:, t:t + 1])
                nc.vector.tensor_mul(b_t, ps_ab[:, 1:2], gamma_sb[:, t:t + 1])
                nc.vector.tensor_sub(b_t, beta_sb[:, t:t + 1], b_t)
                cA.append(a_t)
                cB.append(b_t)

            # q0 = Wq^T B per head-pair tile (before Wq is scaled in place).
            q0_sb = []
            for j in range(PAIRS):
                pq0 = pp.tile([128, 512], F32, tag="proj")
                for t in range(CT):
                    nc.tensor.matmul(
                        pq0[:, 0:1],
                        lhsT=wq_sb[t][:, j * 128:(j + 1) * 128].bitcast(F32),
                        rhs=cB[t],
                        start=(t == 0), stop=(t == CT - 1),
                    )
                q0 = spool.tile([128, 1], F32, tag=f"q0{j}")
                nc.vector.tensor_copy(out=q0, in_=pq0[:, 0:1])
                q0_sb.append(q0)
            # Fold A into Wq rows in place (q0 above already consumed raw Wq).
            for t in range(CT):
                nc.vector.tensor_scalar_mul(
                    out=wq_sb[t], in0=wq_sb[t], scalar1=cA[t]
                )

            # ---- V = context @ Wv  (per ctx row-tile, all heads; +ones col) ----
            # PSUM evacuation on the scalar engine (one strided Copy per tile).
            v_sb = []
            for m in range(MT):
                pv = pp.tile([128, 512], F32, tag="proj")
                for kk in range(KT):
                    nc.tensor.matmul(
                        pv,
                        lhsT=(ctx_sb[kk][:, m * 128:(m + 1) * 128]),
                        rhs=(wv_sb[kk]),
                        start=(kk == 0), stop=(kk == KT - 1),
                    )
                vt = vpool.tile([128, HEADS, HD + 1], BF16, tag=f"v{m}")
                nc.scalar.activation(
                    vt[:, :, 0:HD],
                    pv.rearrange("p (h d) -> p h d", h=HEADS),
                    AF.Copy,
                )
                nc.vector.tensor_copy(
                    out=vt[:, :, HD:HD + 1],
                    in_=ones8_sb.rearrange("p (f one) -> p f one", one=1),
                )
                v_sb.append(vt)

            # ---- per head-pair: K^T and q^T projections, then attention ----
            # Interleaving the projections with the attention keeps the tensor
            # engine busy while the scalar engine works through the exps.
            ot_sb = [
                otpool.tile([128, NL], F32R, tag=f"ot{j}", name=f"ot{j}")
                for j in range(PAIRS)
            ]
            for j in range(PAIRS):
                # K^T_j = (context @ Wk)^T for this head pair
                kt_ = ktpool.tile([128, CTX], BF16, tag=f"kT{j}")
                for cc in range(2):
                    pk = pp.tile([128, 512], F32, tag="proj")
                    for kk in range(KT):
                        nc.tensor.matmul(
                            pk,
                            lhsT=(wk_sb[kk][:, j * 128:(j + 1) * 128]),
                            rhs=(ctx_sb[kk][:, cc * 512:(cc + 1) * 512]),
                            start=(kk == 0), stop=(kk == KT - 1),
                        )
                    nc.scalar.activation(
                        kt_[:, cc * 512:(cc + 1) * 512], pk, AF.Copy
                    )

                # q^T_j (GroupNorm pre-folded; + rank-1 offset q0 as bias)
                qt_ = qpool.tile([128, NL], BF16, tag=f"qT{j}")
                for ncc in range(2):
                    pq = pp.tile([128, 512], F32, tag="proj")
                    for t in range(CT):
                        nc.tensor.matmul(
                            pq,
                            lhsT=(wq_sb[t][:, j * 128:(j + 1) * 128]),
                            rhs=(x_sb[t][:, ncc * 512:(ncc + 1) * 512]),
                            start=(t == 0), stop=(t == CT - 1),
                        )
                    nc.scalar.activation(
                        qt_[:, ncc * 512:(ncc + 1) * 512], pq, AF.Identity,
                        bias=q0_sb[j],
                    )

                # attention for both 512-token chunks of this pair.
                # Scores for both heads of a pair are issued back-to-back with
                # tile_position (0,0)/(64,0): K=64 matmuls on distinct PE array
                # row-groups run concurrently (~2x). One exp covers both heads.
                for ncc in range(2):
                    cs = slice(ncc * 512, (ncc + 1) * 512)
                    e_tiles = []
                    for m in range(MT):
                        stp = pst.tile([128, 1024], F32, tag="st")
                        nc.tensor.matmul(
                            stp[:, 0:512],
                            lhsT=kt_[0:HD, m * 128:(m + 1) * 128],
                            rhs=qt_[0:HD, cs],
                            start=True, stop=True,
                        )
                        nc.tensor.matmul(
                            stp[:, 512:1024],
                            lhsT=kt_[HD:128, m * 128:(m + 1) * 128],
                            rhs=qt_[HD:128, cs],
                            start=True, stop=True,
                        )
                        et = epool.tile([128, 1024], BF16, tag="e")
                        nc.scalar.activation(et, stp, AF.Exp, scale=SCALE)
                        e_tiles.append(et)
                    for half in range(2):
                        h = 2 * j + half
                        rs = slice(half * HD, (half + 1) * HD)
                        es = slice(half * 512, (half + 1) * 512)
                        avp = pav.tile([HD + 1, 512], F32, tag="av")
                        for m in range(MT):
                            nc.tensor.matmul(
                                avp,
                                lhsT=v_sb[m][:, h, :],
                                rhs=e_tiles[m][:, es],
                                start=(m == 0), stop=(m == MT - 1),
                            )
                        rec1 = dpool.tile([1, 512], F32, tag="rec1")
                        nc.vector.reciprocal(out=rec1, in_=avp[HD:HD + 1, :])
                        nc.sync.dma_start(out=den_d[h, ncc, :], in_=rec1)
                        den_row = den_d[h, ncc, :]
                        den_bc_ap = bass.AP(
                            tensor=den_row.tensor,
                            offset=den_row.offset,
                            ap=[[0, HD], [1, 512]],
                        )
                        rec = dpool.tile([HD, 512], F32, tag="rec")
                        nc.sync.dma_start(out=rec, in_=den_bc_ap)
                        nc.vector.tensor_mul(ot_sb[j][rs, cs], avp[0:HD, :], rec)

            # ---- out = OT^T Wo + bo + residual ----
            for ncc in range(2):
                cs = slice(ncc * 512, (ncc + 1) * 512)
                for t in range(CT):
                    po = pp.tile([128, 512], F32, tag="proj")
                    for jj in range(PAIRS):
                        nc.tensor.matmul(
                            po,
                            lhsT=(wo_sb[jj][:, t * 128:(t + 1) * 128]),
                            rhs=(ot_sb[jj][:, cs]),
                            start=(jj == 0), stop=(jj == PAIRS - 1),
                        )
                    res = opool.tile([128, 512], F32, tag="res")
                    nc.vector.scalar_tensor_tensor(
                        out=res,
                        in0=po,
                        scalar=bo_sb[:, t:t + 1],
                        in1=x_sb[t][:, cs],
                        op0=ALU.add,
                        op1=ALU.add,
                    )
                    nc.sync.dma_start(out=out_d[t, :, cs], in_=res)

    nc.finalize()
    return nc


def _host_prep(x, context, gamma, beta, Wq, Wk, Wv, Wo, bo):
    """Build the 8 per-core input maps (host-side slicing/transposes only)."""
    x2 = np.ascontiguousarray(x, np.float32).reshape(B, C, N)
    ctx = np.ascontiguousarray(context, np.float32)

    sel_fwd = np.zeros((128, 8), np.float32)
    for p in range(128):
        sel_fwd[p, p // 16] = 1.0 / 64.0  # 16 channels x 4 cores
    sel_bwd = np.zeros((8, 128), np.float32)
    for p in range(128):
        sel_bwd[p // 16, p] = 1.0

    shared = {
        "wq": np.ascontiguousarray(Wq, np.float32).reshape(CT, 128, INNER),
        "wk": np.ascontiguousarray(Wk, np.float32).reshape(KT, 128, INNER),
        "wv": np.ascontiguousarray(Wv, np.float32).reshape(KT, 128, INNER),
        "wo": np.ascontiguousarray(Wo, np.float32).reshape(CT, 128, C),
        "gamma_t": np.ascontiguousarray(
            np.asarray(gamma, np.float32).reshape(CT, 128).T
        ),
        "beta_t": np.ascontiguousarray(
            np.asarray(beta, np.float32).reshape(CT, 128).T
        ),
        "bo_t": np.ascontiguousarray(np.asarray(bo, np.float32).reshape(CT, 128).T),
        "sel_fwd": sel_fwd,
        "sel_bwd": sel_bwd,
        "stats_in": np.zeros((8, 8), np.float32),
        "stats_out": np.zeros((8, 8), np.float32),
    }

    in_maps = []
    for core in range(8):
        b, qt = core // 4, core % 4
        m = dict(shared)
        m["x_l"] = np.ascontiguousarray(
            x2[b, :, qt * NL:(qt + 1) * NL]
        ).reshape(CT, 128, NL)
        m["ctxT"] = np.ascontiguousarray(ctx[b].T).reshape(KT, 128, CTX)
        in_maps.append(m)
    return in_maps


def _assemble(results):
    out = np.zeros((B, C, N), np.float32)
    for core in range(8):
        b, qt = core // 4, core % 4
        out[b, :, qt * NL:(qt + 1) * NL] = results[core]["out_l"].reshape(C, NL)
    return out.reshape(B, C, 16, 16, 16)


def run(inputs, trace=False):
    global _CACHED_NC
    if _CACHED_NC is None:
        _CACHED_NC = build_nc()
    nc = _CACHED_NC
    in_maps = _host_prep(**inputs)
    # stats_in/stats_out are internal dram tensors, not ExternalInputs
    for m in in_maps:
        m.pop("stats_in")
        m.pop("stats_out")
    bkr = bass_utils.run_bass_kernel_spmd(
        nc, in_maps, list(range(8)), trace=trace
    )
    return _assemble(bkr.results), bkr


def kernel(**inputs):
    out, _ = run(inputs)
    return out


# revision 7
# speedup vs baseline: 22.1553x; 1.2599x over previous
"""Trainium2 Bass kernel for a GroupNorm + cross-attention block.

Reference computation (shapes hardcoded):
  x:[2,512,16,16,16] -> GroupNorm(32 groups over (16ch x 4096 spatial))
  q = xn_seq @ Wq ; k,v = context @ Wk/Wv  (context:[2,1024,768])
  attn = softmax(q k^T / 8) ; out = (attn v) @ Wo + bo + residual
  output: [2,512,16,16,16]

Sharding: 8 cores = 2 batches x 4 sequence-quarters (1024 of 4096 voxel
tokens each). Each core computes all 8 heads for its token slice; the only
cross-core communication is a [8,8] f32 AllReduce of GroupNorm statistics
within each 4-core batch group.

Device layout notes:
 - Everything keeps channels/inner-dim on the partition axis, so no
   transposes are needed anywhere on device: the host feeds context^T.
 - GroupNorm is folded into the q projection: Wq rows are scaled by the
   per-channel A = gamma*rstd and a rank-1 offset q0 = Wq^T B is added,
   so normalized x is never materialized.
 - Softmax runs without max-subtraction (scores are O(+-6) here, safe in
   fp32) with exp on the scalar engine, reading score PSUM directly.
 - The softmax denominator comes from an extra all-ones column appended to
   the V tile (lhsT [128,65]) so row 64 of the AV matmul accumulates
   sum(exp) for free.
 - x and Wq stay float32r (full PE speed at >=256 free dim, ~fp32
   accuracy); context/Wk/Wv/Wo are fed as bf16 from the host, halving
   their DMA traffic at the same PE speed.

Scheduling notes (the Tile scheduler prioritizes by emission order and a
pool tag's buffers are a ring in allocation order, so both are chosen
deliberately):
 - Input DMAs ordered x, ctx, wk, wq, wv, wo: x starts the GroupNorm stats
   chain (the longest latency chain), ctx+wk+wq unblock the first
   head-pair's K^T/q^T projections ~15us in; wv/wo stream in behind them.
 - The GroupNorm stats DMA round-trip runs on the scalar-engine HWDGE ring:
   on the sync ring it would queue FIFO behind all ~7MB of input loads.
 - The tiny stats/coefficient/q0 matmuls allocate their PSUM from the
   scores pool's tag ring (they are done before scores start), keeping the
   projection pool's 2-buffer ring free for K^T/q^T: otherwise the first
   K^T tile's PSUM allocation waits on the stats chain.
 - A dummy Ln at t=0 preloads the ln/exp activation table set so the
   ~1.3us load is off the stats critical chain (Exp pays one on-chain load;
   the 64 softmax exps then hit a warm table).
 - K^T/q^T projections and scores+exp are interleaved PER HEAD-PAIR, and
   each pair's AV matmuls are emitted after the NEXT pair's scores: exp is
   the scarce resource (64us of scalar-engine time), so score production
   outranks AV work on the tensor engine whenever both are ready.
"""

import os
from contextlib import ExitStack, nullcontext

import numpy as np

import concourse.bass as bass
import concourse.mybir as mybir
import concourse.tile as tile
from concourse import bacc, bass_utils

F32 = mybir.dt.float32
F32R = mybir.dt.float32r
BF16 = mybir.dt.bfloat16
AF = mybir.ActivationFunctionType
ALU = mybir.AluOpType

B = 2
C = 512
N = 4096            # voxel tokens per batch (16*16*16)
NL = 1024           # tokens per core (N / 4)
CTX = 1024
CTXD = 768
HEADS = 8
HD = 64
INNER = HEADS * HD  # 512
GROUPS = 32
EPS = 1e-5
SCALE = HD ** -0.5

CT = C // 128       # 4 channel tiles
KT = CTXD // 128    # 6 context-dim tiles
MT = CTX // 128     # 8 ctx row tiles
PAIRS = HEADS // 2  # 4 head-pair tiles (128 rows each)

_CACHED_NC = None


def build_nc(loop_iters=1, skip_collective=False):
    # Bacc (not raw Bass): its finalize() runs the wait-splitting passes
    # (move_matmul_waits_to_ldweights / generate_event_semaphores) that the
    # TRN2 ISA requires — walrus rejects multi-wait matmuls otherwise.
    # loop_iters > 1 wraps the body in a device-side For_i so per-iteration
    # device time can be measured without per-dispatch overhead.
    nc = bacc.Bacc("TRN2", target_bir_lowering=False, debug=False, num_devices=8)

    x_d = nc.dram_tensor("x_l", [CT, 128, NL], F32R, kind="ExternalInput")
    ctxT_d = nc.dram_tensor("ctxT", [KT, 128, CTX], BF16, kind="ExternalInput")
    wq_d = nc.dram_tensor("wq", [CT, 128, INNER], F32R, kind="ExternalInput")
    wk_d = nc.dram_tensor("wk", [KT, 128, INNER], BF16, kind="ExternalInput")
    wv_d = nc.dram_tensor("wv", [KT, 128, INNER], BF16, kind="ExternalInput")
    wo_d = nc.dram_tensor("wo", [CT, 128, C], BF16, kind="ExternalInput")
    gamma_d = nc.dram_tensor("gamma_t", [128, CT], F32, kind="ExternalInput")
    beta_d = nc.dram_tensor("beta_t", [128, CT], F32, kind="ExternalInput")
    bo_d = nc.dram_tensor("bo_t", [128, CT], F32, kind="ExternalInput")
    self_fwd_d = nc.dram_tensor("sel_fwd", [128, 8], F32, kind="ExternalInput")
    sel_bwd_d = nc.dram_tensor("sel_bwd", [8, 128], F32, kind="ExternalInput")
    out_d = nc.dram_tensor("out_l", [CT, 128, NL], F32, kind="ExternalOutput")

    stats_in_d = nc.dram_tensor("stats_in", [8, 8], F32)
    stats_out_d = nc.dram_tensor("stats_out", [8, 8], F32)
    # scratch for partition-broadcasting softmax denominators (DRAM bounce:
    # SBUF/PSUM sources cannot be read with partition-stride 0, DRAM can)
    den_d = nc.dram_tensor("den_scratch", [HEADS, 2, 512], F32)

    with tile.TileContext(nc) as tc, ExitStack() as ctx:
        consts = ctx.enter_context(tc.tile_pool(name="consts", bufs=1))
        wpool = ctx.enter_context(tc.tile_pool(name="weights", bufs=1))
        xpool = ctx.enter_context(tc.tile_pool(name="x", bufs=1))
        cxpool = ctx.enter_context(tc.tile_pool(name="ctx", bufs=1))
        ktpool = ctx.enter_context(tc.tile_pool(name="kt", bufs=1))
        vpool = ctx.enter_context(tc.tile_pool(name="v", bufs=1))
        qpool = ctx.enter_context(tc.tile_pool(name="qt", bufs=1))
        epool = ctx.enter_context(tc.tile_pool(name="e", bufs=36))
        otpool = ctx.enter_context(tc.tile_pool(name="ot", bufs=1))
        spool = ctx.enter_context(tc.tile_pool(name="small", bufs=4))
        dpool = ctx.enter_context(tc.tile_pool(name="den", bufs=4))
        opool = ctx.enter_context(tc.tile_pool(name="outs", bufs=3))

        pp = ctx.enter_context(tc.tile_pool(name="pproj", bufs=2, space="PSUM"))
        pst = ctx.enter_context(tc.tile_pool(name="pst", bufs=2, space="PSUM"))
        pav = ctx.enter_context(tc.tile_pool(name="pav", bufs=2, space="PSUM"))

        loop_cm = tc.For_i(0, loop_iters, 1) if loop_iters > 1 else nullcontext()
        with loop_cm:

            # ---- input loads, ordered by first use ----
            # x goes before even the tiny const loads: it heads the longest
            # dependency chain (stats -> qT), and each const DMA in front of
            # it costs ~0.5us of queue head-of-line latency.
            x_sb = []
            for t in range(CT):
                xt = xpool.tile([128, NL], F32R, tag=f"x{t}")
                nc.sync.dma_start(out=xt, in_=x_d[t])
                x_sb.append(xt)
            sel_fwd = consts.tile([128, 8], F32, tag="sel_fwd")
            nc.sync.dma_start(out=sel_fwd, in_=self_fwd_d[:, :])
            sel_bwd = consts.tile([8, 128], F32, tag="sel_bwd")
            nc.sync.dma_start(out=sel_bwd, in_=sel_bwd_d[:, :])
            gamma_sb = consts.tile([128, CT], F32, tag="gamma")
            nc.sync.dma_start(out=gamma_sb, in_=gamma_d[:, :])
            beta_sb = consts.tile([128, CT], F32, tag="beta")
            nc.sync.dma_start(out=beta_sb, in_=beta_d[:, :])
            bo_sb = consts.tile([128, CT], F32, tag="bo")
            nc.sync.dma_start(out=bo_sb, in_=bo_d[:, :])
            eps_sb = consts.tile([128, 1], F32, tag="eps")
            nc.vector.memset(eps_sb, EPS)
            ones8_sb = consts.tile([128, 8], BF16, tag="ones8sb")
            nc.vector.memset(ones8_sb, 1.0)
            # preload the ln/exp activation table set while DMAs stream: Ln
            # comes first dynamically (stats chain), so warm with Ln.
            warm_sb = consts.tile([128, 1], F32, tag="actwarm")
            nc.scalar.activation(warm_sb, eps_sb, AF.Ln)
            ctx_sb = []
            for kk in range(KT):
                ct_ = cxpool.tile([128, CTX], BF16, tag=f"ctx{kk}")
                nc.sync.dma_start(out=ct_, in_=ctxT_d[kk])
                ctx_sb.append(ct_)
            wk_sb = []
            for kk in range(KT):
                w = wpool.tile([128, INNER], BF16, tag=f"wk{kk}")
                nc.sync.dma_start(out=w, in_=wk_d[kk])
                wk_sb.append(w)
            wq_sb = []
            for t in range(CT):
                w = wpool.tile([128, INNER], F32R, tag=f"wq{t}")
                nc.sync.dma_start(out=w, in_=wq_d[t])
                wq_sb.append(w)
            wv_sb = []
            for kk in range(KT):
                w = wpool.tile([128, INNER], BF16, tag=f"wv{kk}")
                nc.sync.dma_start(out=w, in_=wv_d[kk])
                wv_sb.append(w)
            wo_sb = []
            for t in range(CT):
                w = wpool.tile([128, C], BF16, tag=f"wo{t}")
                nc.sync.dma_start(out=w, in_=wo_d[t])
                wo_sb.append(w)

            # ---- GroupNorm statistics ----
            # per-channel (mean, E[x^2]) over the local token slice, group-reduced
            # on the PE with sel_fwd (value 1/64: 16 channels x 4 cores), then
            # AllReduced within the batch group. The tiny matmuls borrow PSUM
            # from the scores tag ring (see scheduling notes above).
            ps_stats = pst.tile([128, 1024], F32, tag="st")
            for t in range(CT):
                st6 = spool.tile([128, 2, 6], F32, tag="bn6")
                for sg in range(2):
                    nc.vector.bn_stats(
                        out=st6[:, sg, :], in_=x_sb[t][:, sg * 512:(sg + 1) * 512]
                    )
                mv = spool.tile([128, 2], F32, tag="mv")
                nc.vector.bn_aggr(out=mv, in_=st6)
                s12 = spool.tile([128, 2], F32, tag="s12")
                nc.vector.tensor_copy(out=s12[:, 0:1], in_=mv[:, 0:1])
                nc.vector.tensor_mul(s12[:, 1:2], mv[:, 0:1], mv[:, 0:1])
                nc.vector.tensor_add(s12[:, 1:2], s12[:, 1:2], mv[:, 1:2])
                nc.tensor.matmul(
                    ps_stats[0:8, t * 2:t * 2 + 2], lhsT=sel_fwd, rhs=s12,
                    start=True, stop=True,
                )
            stats_sb = spool.tile([8, 8], F32, tag="gst")
            nc.vector.tensor_copy(out=stats_sb, in_=ps_stats[0:8, 0:8])
            # stats round-trip on the scalar-engine HWDGE ring (the sync ring
            # is busy streaming the input loads; this one is empty).
            nc.scalar.dma_start(out=stats_in_d[:, :], in_=stats_sb)
            if skip_collective:
                # timing-only variant: collectives inside a device-side For_i
                # desync the mesh on the 2nd iteration, so the timing loop
                # substitutes a local DRAM copy (output values are wrong by a
                # constant stats factor; latency profile is comparable).
                nc.scalar.dma_start(out=stats_out_d[:, :], in_=stats_in_d[:, :])
            else:
                nc.gpsimd.collective_compute(
                    "AllReduce",
                    ALU.add,
                    replica_groups=[[0, 1, 2, 3], [4, 5, 6, 7]],
                    ins=[stats_in_d[:, :]],
                    outs=[stats_out_d[:, :]],
                )
            g_sb = spool.tile([8, 8], F32, tag="gout")
            nc.scalar.dma_start(out=g_sb, in_=stats_out_d[:, :])

            # Per channel-tile affine coefficients:
            #   A = gamma * rstd,  B = beta - mean*rstd*gamma
            # rstd = exp(-0.5 * ln(var + eps)); the Exp here pays the one
            # on-chain table load, after which the 64 softmax exps are warm.
            cA, cB = [], []
            for t in range(CT):
                m1 = g_sb[:, 2 * t:2 * t + 1]
                m2 = g_sb[:, 2 * t + 1:2 * t + 2]
                var8 = spool.tile([8, 1], F32, tag=f"var{t}")
                nc.vector.tensor_mul(var8, m1, m1)
                nc.vector.tensor_sub(var8, m2, var8)
                nc.scalar.activation(var8, var8, AF.Ln, bias=eps_sb[0:8, :])
                nc.scalar.activation(var8, var8, AF.Exp, scale=-0.5)  # rstd
                ab8 = spool.tile([8, 2], F32, tag=f"ab{t}")
                nc.vector.tensor_copy(out=ab8[:, 0:1], in_=var8)
                nc.vector.tensor_mul(ab8[:, 1:2], m1, var8)  # mean*rstd
                ps_ab = pst.tile([128, 1024], F32, tag="st")
                nc.tensor.matmul(
                    ps_ab[:, 0:2], lhsT=sel_bwd, rhs=ab8, start=True, stop=True
                )
                a_t = spool.tile([128, 1], F32, tag=f"cA{t}")
                b_t = spool.tile([128, 1], F32, tag=f"cB{t}")
                nc.vector.tensor_mul(a_t, ps_ab[:, 0:1], gamma_sb[:, t:t + 1])
                nc.vector.tensor_mul(b_t, ps_ab[:, 1:2], gamma_sb[:, t:t + 1])
                nc.vector.tensor_sub(b_t, beta_sb[:, t:t + 1], b_t)
                cA.append(a_t)
                cB.append(b_t)

            # q0 = Wq^T B per head-pair tile (before Wq is scaled in place).
            q0_sb = []
            for j in range(PAIRS):
                pq0 = pst.tile([128, 1024], F32, tag="st")
                for t in range(CT):
                    nc.tensor.matmul(
                        pq0[:, 0:1],
                        lhsT=wq_sb[t][:, j * 128:(j + 1) * 128].bitcast(F32),
                        rhs=cB[t],
                        start=(t == 0), stop=(t == CT - 1),
                    )
                q0 = spool.tile([128, 1], F32, tag=f"q0{j}")
                nc.vector.tensor_copy(out=q0, in_=pq0[:, 0:1])
                q0_sb.append(q0)
            # Fold A into Wq rows in place (q0 above already consumed raw
            # Wq), per head-pair slice just before that pair's qT projection:
            # only pair 0's four slices sit on the critical chain. On the
            # scalar engine (Copy is in the loaded table set, and the vector
            # engine is busy with K^T evacuations around this time).
            def scale_wq(j):
                js = slice(j * 128, (j + 1) * 128)
                for t in range(CT):
                    nc.scalar.activation(
                        wq_sb[t][:, js], wq_sb[t][:, js], AF.Copy, scale=cA[t]
                    )

            ot_sb = [
                otpool.tile([128, NL], BF16, tag=f"ot{j}", name=f"ot{j}")
                for j in range(PAIRS)
            ]
            v_sb = []

            def emit_kq(j):
                # K^T_j = (context @ Wk)^T and q^T_j for one head pair.
                kt_ = ktpool.tile([128, CTX], BF16, tag=f"kT{j}")
                for cc in range(2):
                    pk = pp.tile([128, 512], F32, tag="proj")
                    for kk in range(KT):
                        nc.tensor.matmul(
                            pk,
                            lhsT=(wk_sb[kk][:, j * 128:(j + 1) * 128]),
                            rhs=(ctx_sb[kk][:, cc * 512:(cc + 1) * 512]),
                            start=(kk == 0), stop=(kk == KT - 1),
                        )
                    nc.vector.tensor_copy(
                        out=kt_[:, cc * 512:(cc + 1) * 512], in_=pk
                    )
                scale_wq(j)
                qt_ = qpool.tile([128, NL], BF16, tag=f"qT{j}")
                for ncc in range(2):
                    pq = pp.tile([128, 512], F32, tag="proj")
                    for t in range(CT):
                        nc.tensor.matmul(
                            pq,
                            lhsT=(wq_sb[t][:, j * 128:(j + 1) * 128]),
                            rhs=(x_sb[t][:, ncc * 512:(ncc + 1) * 512]),
                            start=(t == 0), stop=(t == CT - 1),
                        )
                    nc.vector.tensor_scalar_add(
                        out=qt_[:, ncc * 512:(ncc + 1) * 512],
                        in0=pq,
                        scalar1=q0_sb[j],
                    )
                return kt_, qt_

            def emit_scores(j, kt_, qt_, ncc):
                # Scores for both heads of a pair are issued back-to-back with
                # tile_position (0,0)/(64,0): K=64 matmuls on distinct PE array
                # row-groups run concurrently (~2x). One exp covers both heads.
                cs = slice(ncc * 512, (ncc + 1) * 512)
                e_tiles = []
                for m in range(MT):
                    stp = pst.tile([128, 1024], F32, tag="st")
                    nc.tensor.matmul(
                        stp[:, 0:512],
                        lhsT=kt_[0:HD, m * 128:(m + 1) * 128],
                        rhs=qt_[0:HD, cs],
                        start=True, stop=True,
                    )
                    nc.tensor.matmul(
                        stp[:, 512:1024],
                        lhsT=kt_[HD:128, m * 128:(m + 1) * 128],
                        rhs=qt_[HD:128, cs],
                        start=True, stop=True,
                    )
                    et = epool.tile([128, 1024], BF16, tag="e")
                    nc.scalar.activation(et, stp, AF.Exp, scale=SCALE)
                    e_tiles.append(et)
                return e_tiles

            ones64_sb = consts.tile([1, HD], F32, tag="ones64")
            nc.vector.memset(ones64_sb, 1.0)

            def emit_av(j, ncc, e_tiles, fast_bc=False):
                cs = slice(ncc * 512, (ncc + 1) * 512)
                for half in range(2):
                    h = 2 * j + half
                    rs = slice(half * HD, (half + 1) * HD)
                    es = slice(half * 512, (half + 1) * 512)
                    avp = pav.tile([HD + 1, 512], F32, tag="av")
                    for m in range(MT):
                        nc.tensor.matmul(
                            avp,
                            lhsT=v_sb[m][:, h, :],
                            rhs=e_tiles[m][:, es],
                            start=(m == 0), stop=(m == MT - 1),
                        )
                    rec1 = dpool.tile([1, 512], F32, tag="rec1")
                    nc.vector.reciprocal(out=rec1, in_=avp[HD:HD + 1, :])
                    rec = dpool.tile([HD, 512], F32, tag="rec")
                    if fast_bc:
                        # tail path: broadcast 1/den across partitions with a
                        # K=1 matmul + PSUM evacuation (~2us) instead of the
                        # ~5us DRAM bounce; borrows the idle scores PSUM ring.
                        bc_ps = pst.tile([128, 1024], F32, tag="st")
                        nc.tensor.matmul(
                            bc_ps[0:HD, 0:512], lhsT=ones64_sb, rhs=rec1,
                            start=True, stop=True,
                        )
                        nc.vector.tensor_copy(out=rec, in_=bc_ps[0:HD, 0:512])
                    else:
                        # steady state: DRAM bounce (SBUF/PSUM can't be read
                        # with partition-stride 0, DRAM can); uses the idle
                        # DMA engines instead of the busy vector engine.
                        nc.sync.dma_start(out=den_d[h, ncc, :], in_=rec1)
                        den_row = den_d[h, ncc, :]
                        den_bc_ap = bass.AP(
                            tensor=den_row.tensor,
                            offset=den_row.offset,
                            ap=[[0, HD], [1, 512]],
                        )
                        nc.sync.dma_start(out=rec, in_=den_bc_ap)
                    nc.vector.tensor_mul(ot_sb[j][rs, cs], avp[0:HD, :], rec)

            def emit_v():
                # V = context @ Wv (per ctx row-tile, all heads; +ones col)
                for m in range(MT):
                    pv = pp.tile([128, 512], F32, tag="proj")
                    for kk in range(KT):
                        nc.tensor.matmul(
                            pv,
                            lhsT=(ctx_sb[kk][:, m * 128:(m + 1) * 128]),
                            rhs=(wv_sb[kk]),
                            start=(kk == 0), stop=(kk == KT - 1),
                        )
                    vt = vpool.tile([128, HEADS, HD + 1], BF16, tag=f"v{m}")
                    nc.vector.tensor_copy(
                        out=vt[:, :, 0:HD],
                        in_=pv.rearrange("p (h d) -> p h d", h=HEADS),
                    )
                    nc.vector.tensor_copy(
                        out=vt[:, :, HD:HD + 1],
                        in_=ones8_sb.rearrange("p (f one) -> p f one", one=1),
                    )
                    v_sb.append(vt)

            # Emission order = tensor-engine priority. Scores (which feed the
            # scalar engine's 64us exp stream) outrank V and AV; V sits after
            # pair 1's scores so those exps keep the scalar engine fed while
            # the PE chews through V's 10us block.
            kq = {0: emit_kq(0)}
            e_saved = {(0, 0): emit_scores(0, kq[0][0], kq[0][1], 0),
                       (0, 1): emit_scores(0, kq[0][0], kq[0][1], 1)}
            emit_v()
            kq[1] = emit_kq(1)
            for j in range(1, PAIRS):
                for ncc in range(2):
                    e_saved[(j, ncc)] = emit_scores(j, kq[j][0], kq[j][1], ncc)
                if j + 1 < PAIRS:
                    kq[j + 1] = emit_kq(j + 1)
                for ncc in range(2):
                    emit_av(j - 1, ncc, e_saved.pop((j - 1, ncc)))
            for ncc in range(2):
                emit_av(PAIRS - 1, ncc, e_saved.pop((PAIRS - 1, ncc)),
                        fast_bc=True)

            # ---- out = OT^T Wo + bo + residual ----
            for ncc in range(2):
                cs = slice(ncc * 512, (ncc + 1) * 512)
                for t in range(CT):
                    po = pp.tile([128, 512], F32, tag="proj")
                    for jj in range(PAIRS):
                        nc.tensor.matmul(
                            po,
                            lhsT=(wo_sb[jj][:, t * 128:(t + 1) * 128]),
                            rhs=(ot_sb[jj][:, cs]),
                            start=(jj == 0), stop=(jj == PAIRS - 1),
                        )
                    res = opool.tile([128, 512], F32, tag="res")
                    nc.vector.scalar_tensor_tensor(
                        out=res,
                        in0=po,
                        scalar=bo_sb[:, t:t + 1],
                        in1=x_sb[t][:, cs],
                        op0=ALU.add,
                        op1=ALU.add,
                    )
                    nc.sync.dma_start(out=out_d[t, :, cs], in_=res)

    nc.finalize()
    return nc


def _host_prep(x, context, gamma, beta, Wq, Wk, Wv, Wo, bo):
    """Build the 8 per-core input maps (host-side slicing/transposes only)."""
    x2 = np.ascontiguousarray(x, np.float32).reshape(B, C, N)
    ctx = np.ascontiguousarray(context, np.float32)

    sel_fwd = np.zeros((128, 8), np.float32)
    for p in range(128):
        sel_fwd[p, p // 16] = 1.0 / 64.0  # 16 channels x 4 cores
    sel_bwd = np.zeros((8, 128), np.float32)
    for p in range(128):
        sel_bwd[p // 16, p] = 1.0

    def _bf16(a):
        import ml_dtypes
        return np.ascontiguousarray(a).astype(ml_dtypes.bfloat16)

    shared = {
        "wq": np.ascontiguousarray(Wq, np.float32).reshape(CT, 128, INNER),
        "wk": _bf16(np.asarray(Wk, np.float32).reshape(KT, 128, INNER)),
        "wv": _bf16(np.asarray(Wv, np.float32).reshape(KT, 128, INNER)),
        "wo": _bf16(np.asarray(Wo, np.float32).reshape(CT, 128, C)),
        "gamma_t": np.ascontiguousarray(
            np.asarray(gamma, np.float32).reshape(CT, 128).T
        ),
        "beta_t": np.ascontiguousarray(
            np.asarray(beta, np.float32).reshape(CT, 128).T
        ),
        "bo_t": np.ascontiguousarray(np.asarray(bo, np.float32).reshape(CT, 128).T),
        "sel_fwd": sel_fwd,
        "sel_bwd": sel_bwd,
        "stats_in": np.zeros((8, 8), np.float32),
        "stats_out": np.zeros((8, 8), np.float32),
    }

    in_maps = []
    for core in range(8):
        b, qt = core // 4, core % 4
        m = dict(shared)
        m["x_l"] = np.ascontiguousarray(
            x2[b, :, qt * NL:(qt + 1) * NL]
        ).reshape(CT, 128, NL)
        m["ctxT"] = _bf16(ctx[b].T.reshape(KT, 128, CTX))
        in_maps.append(m)
    return in_maps


def _assemble(results):
    out = np.zeros((B, C, N), np.float32)
    for core in range(8):
        b, qt = core // 4, core % 4
        out[b, :, qt * NL:(qt + 1) * NL] = results[core]["out_l"].reshape(C, NL)
    return out.reshape(B, C, 16, 16, 16)


def run(inputs, trace=False):
    global _CACHED_NC
    if _CACHED_NC is None:
        _CACHED_NC = build_nc()
    nc = _CACHED_NC
    in_maps = _host_prep(**inputs)
    # stats_in/stats_out are internal dram tensors, not ExternalInputs
    for m in in_maps:
        m.pop("stats_in")
        m.pop("stats_out")
    bkr = bass_utils.run_bass_kernel_spmd(
        nc, in_maps, list(range(8)), trace=trace
    )
    return _assemble(bkr.results), bkr


def kernel(**inputs):
    out, _ = run(inputs)
    return out


# revision 14
# speedup vs baseline: 22.8081x; 1.0295x over previous
"""Trainium2 Bass kernel for a GroupNorm + cross-attention block.

Reference computation (shapes hardcoded):
  x:[2,512,16,16,16] -> GroupNorm(32 groups over (16ch x 4096 spatial))
  q = xn_seq @ Wq ; k,v = context @ Wk/Wv  (context:[2,1024,768])
  attn = softmax(q k^T / 8) ; out = (attn v) @ Wo + bo + residual
  output: [2,512,16,16,16]

Sharding: 8 cores = 2 batches x 4 sequence-quarters (1024 of 4096 voxel
tokens each). Each core computes all 8 heads for its token slice; the only
cross-core communication is a [8,8] f32 AllReduce of GroupNorm statistics
within each 4-core batch group.

Device layout notes:
 - Everything keeps channels/inner-dim on the partition axis, so no
   transposes are needed anywhere on device: the host feeds context^T.
 - GroupNorm is folded into the q projection: Wq rows are scaled by the
   per-channel A = gamma*rstd and a rank-1 offset q0 = Wq^T B is added,
   so normalized x is never materialized.
 - Softmax runs without max-subtraction (scores are O(+-6) here, safe in
   fp32) with exp on the scalar engine, reading score PSUM directly.
 - The softmax denominator comes from an extra all-ones column appended to
   the V tile (lhsT [128,65]) so row 64 of the AV matmul accumulates
   sum(exp) for free.
 - x and Wq stay float32r (full PE speed at >=256 free dim, ~fp32
   accuracy); context/Wk/Wv/Wo are fed as bf16 from the host, halving
   their DMA traffic at the same PE speed.

Scheduling notes (the Tile scheduler prioritizes by emission order and a
pool tag's buffers are a ring in allocation order, so both are chosen
deliberately):
 - Input DMAs ordered x, ctx, wk, wq, wv, wo: x starts the GroupNorm stats
   chain (the longest latency chain), ctx+wk+wq unblock the first
   head-pair's K^T/q^T projections ~15us in; wv/wo stream in behind them.
 - The GroupNorm stats DMA round-trip runs on the scalar-engine HWDGE ring:
   on the sync ring it would queue FIFO behind all ~7MB of input loads.
 - The tiny stats/coefficient/q0 matmuls allocate their PSUM from the
   scores pool's tag ring (they are done before scores start), keeping the
   projection pool's 2-buffer ring free for K^T/q^T: otherwise the first
   K^T tile's PSUM allocation waits on the stats chain.
 - A dummy Ln at t=0 preloads the ln/exp activation table set so the
   ~1.3us load is off the stats critical chain (Exp pays one on-chain load;
   the 64 softmax exps then hit a warm table).
 - K^T/q^T projections and scores+exp are interleaved PER HEAD-PAIR, and
   each pair's AV matmuls are emitted after the NEXT pair's scores: exp is
   the scarce resource (64us of scalar-engine time), so score production
   outranks AV work on the tensor engine whenever both are ready.
"""

import os
from contextlib import ExitStack, nullcontext

import numpy as np

import concourse.bass as bass
import concourse.mybir as mybir
import concourse.tile as tile
from concourse import bacc, bass_utils

F32 = mybir.dt.float32
F32R = mybir.dt.float32r
BF16 = mybir.dt.bfloat16
AF = mybir.ActivationFunctionType
ALU = mybir.AluOpType

B = 2
C = 512
N = 4096            # voxel tokens per batch (16*16*16)
NL = 1024           # tokens per core (N / 4)
CTX = 1024
CTXD = 768
HEADS = 8
HD = 64
INNER = HEADS * HD  # 512
GROUPS = 32
EPS = 1e-5
SCALE = HD ** -0.5

CT = C // 128       # 4 channel tiles
KT = CTXD // 128    # 6 context-dim tiles
MT = CTX // 128     # 8 ctx row tiles
PAIRS = HEADS // 2  # 4 head-pair tiles (128 rows each)

_CACHED_NC = None


def build_nc(loop_iters=1, skip_collective=False):
    # Bacc (not raw Bass): its finalize() runs the wait-splitting passes
    # (move_matmul_waits_to_ldweights / generate_event_semaphores) that the
    # TRN2 ISA requires — walrus rejects multi-wait matmuls otherwise.
    # loop_iters > 1 wraps the body in a device-side For_i so per-iteration
    # device time can be measured without per-dispatch overhead.
    nc = bacc.Bacc("TRN2", target_bir_lowering=False, debug=False, num_devices=8)

    x_d = nc.dram_tensor("x_l", [CT, 128, NL], F32R, kind="ExternalInput")
    ctxT_d = nc.dram_tensor("ctxT", [KT, 128, CTX], BF16, kind="ExternalInput")
    wq_d = nc.dram_tensor("wq", [CT, 128, INNER], F32R, kind="ExternalInput")
    wk_d = nc.dram_tensor("wk", [KT, 128, INNER], BF16, kind="ExternalInput")
    wv_d = nc.dram_tensor("wv", [KT, 128, INNER], BF16, kind="ExternalInput")
    wo_d = nc.dram_tensor("wo", [CT, 128, C], BF16, kind="ExternalInput")
    gamma_d = nc.dram_tensor("gamma_t", [128, CT], F32, kind="ExternalInput")
    beta_d = nc.dram_tensor("beta_t", [128, CT], F32, kind="ExternalInput")
    bo_d = nc.dram_tensor("bo_t", [128, CT], F32, kind="ExternalInput")
    self_fwd_d = nc.dram_tensor("sel_fwd", [128, 8], F32, kind="ExternalInput")
    sel_bwd_d = nc.dram_tensor("sel_bwd", [8, 128], F32, kind="ExternalInput")
    out_d = nc.dram_tensor("out_l", [CT, 128, NL], F32, kind="ExternalOutput")

    stats_in_d = nc.dram_tensor("stats_in", [8, 8], F32)
    stats_out_d = nc.dram_tensor("stats_out", [8, 8], F32)
    # scratch for partition-broadcasting softmax denominators (DRAM bounce:
    # SBUF/PSUM sources cannot be read with partition-stride 0, DRAM can)
    den_d = nc.dram_tensor("den_scratch", [HEADS, 2, 512], F32)

    with tile.TileContext(nc) as tc, ExitStack() as ctx:
        consts = ctx.enter_context(tc.tile_pool(name="consts", bufs=1))
        wpool = ctx.enter_context(tc.tile_pool(name="weights", bufs=1))
        xpool = ctx.enter_context(tc.tile_pool(name="x", bufs=1))
        cxpool = ctx.enter_context(tc.tile_pool(name="ctx", bufs=1))
        ktpool = ctx.enter_context(tc.tile_pool(name="kt", bufs=1))
        vpool = ctx.enter_context(tc.tile_pool(name="v", bufs=1))
        qpool = ctx.enter_context(tc.tile_pool(name="qt", bufs=1))
        epool = ctx.enter_context(tc.tile_pool(name="e", bufs=36))
        otpool = ctx.enter_context(tc.tile_pool(name="ot", bufs=1))
        spool = ctx.enter_context(tc.tile_pool(name="small", bufs=4))
        dpool = ctx.enter_context(tc.tile_pool(name="den", bufs=4))
        opool = ctx.enter_context(tc.tile_pool(name="outs", bufs=3))

        pp = ctx.enter_context(tc.tile_pool(name="pproj", bufs=2, space="PSUM"))
        pst = ctx.enter_context(tc.tile_pool(name="pst", bufs=2, space="PSUM"))
        pav = ctx.enter_context(tc.tile_pool(name="pav", bufs=2, space="PSUM"))

        loop_cm = tc.For_i(0, loop_iters, 1) if loop_iters > 1 else nullcontext()
        with loop_cm:

            # ---- input loads, ordered by first use ----
            # x goes before even the tiny const loads: it heads the longest
            # dependency chain (stats -> qT), and each const DMA in front of
            # it costs ~0.5us of queue head-of-line latency.
            x_sb = []
            for t in range(CT):
                xt = xpool.tile([128, NL], F32R, tag=f"x{t}")
                nc.sync.dma_start(out=xt, in_=x_d[t])
                x_sb.append(xt)
            sel_fwd = consts.tile([128, 8], F32, tag="sel_fwd")
            nc.sync.dma_start(out=sel_fwd, in_=self_fwd_d[:, :])
            sel_bwd = consts.tile([8, 128], F32, tag="sel_bwd")
            nc.sync.dma_start(out=sel_bwd, in_=sel_bwd_d[:, :])
            gamma_sb = consts.tile([128, CT], F32, tag="gamma")
            nc.sync.dma_start(out=gamma_sb, in_=gamma_d[:, :])
            beta_sb = consts.tile([128, CT], F32, tag="beta")
            nc.sync.dma_start(out=beta_sb, in_=beta_d[:, :])
            bo_sb = consts.tile([128, CT], F32, tag="bo")
            nc.sync.dma_start(out=bo_sb, in_=bo_d[:, :])
            eps_sb = consts.tile([128, 1], F32, tag="eps")
            nc.vector.memset(eps_sb, EPS)
            ones8_sb = consts.tile([128, 8], BF16, tag="ones8sb")
            nc.vector.memset(ones8_sb, 1.0)
            # preload the ln/exp activation table set while DMAs stream: Ln
            # comes first dynamically (stats chain), so warm with Ln.
            warm_sb = consts.tile([128, 1], F32, tag="actwarm")
            nc.scalar.activation(warm_sb, eps_sb, AF.Ln)
            ctx_sb = []
            for kk in range(KT):
                ct_ = cxpool.tile([128, CTX], BF16, tag=f"ctx{kk}")
                nc.sync.dma_start(out=ct_, in_=ctxT_d[kk])
                ctx_sb.append(ct_)
            wk_sb = []
            for kk in range(KT):
                w = wpool.tile([128, INNER], BF16, tag=f"wk{kk}")
                nc.sync.dma_start(out=w, in_=wk_d[kk])
                wk_sb.append(w)
            wq_sb = []
            for t in range(CT):
                w = wpool.tile([128, INNER], F32R, tag=f"wq{t}")
                nc.sync.dma_start(out=w, in_=wq_d[t])
                wq_sb.append(w)
            wv_sb = []
            for kk in range(KT):
                w = wpool.tile([128, INNER], BF16, tag=f"wv{kk}")
                nc.sync.dma_start(out=w, in_=wv_d[kk])
                wv_sb.append(w)
            wo_sb = []
            for t in range(CT):
                w = wpool.tile([128, C], BF16, tag=f"wo{t}")
                nc.sync.dma_start(out=w, in_=wo_d[t])
                wo_sb.append(w)

            # ---- GroupNorm statistics ----
            # per-channel (mean, E[x^2]) over the local token slice, group-reduced
            # on the PE with sel_fwd (value 1/64: 16 channels x 4 cores), then
            # AllReduced within the batch group. The tiny matmuls borrow PSUM
            # from the scores tag ring (see scheduling notes above).
            ps_stats = pst.tile([128, 1024], F32, tag="st")
            for t in range(CT):
                st6 = spool.tile([128, 2, 6], F32, tag="bn6")
                for sg in range(2):
                    nc.vector.bn_stats(
                        out=st6[:, sg, :], in_=x_sb[t][:, sg * 512:(sg + 1) * 512]
                    )
                mv = spool.tile([128, 2], F32, tag="mv")
                nc.vector.bn_aggr(out=mv, in_=st6)
                s12 = spool.tile([128, 2], F32, tag="s12")
                nc.vector.tensor_copy(out=s12[:, 0:1], in_=mv[:, 0:1])
                nc.vector.tensor_mul(s12[:, 1:2], mv[:, 0:1], mv[:, 0:1])
                nc.vector.tensor_add(s12[:, 1:2], s12[:, 1:2], mv[:, 1:2])
                nc.tensor.matmul(
                    ps_stats[0:8, t * 2:t * 2 + 2], lhsT=sel_fwd, rhs=s12,
                    start=True, stop=True,
                )
            stats_sb = spool.tile([8, 8], F32, tag="gst")
            nc.vector.tensor_copy(out=stats_sb, in_=ps_stats[0:8, 0:8])
            # stats round-trip on the scalar-engine HWDGE ring (the sync ring
            # is busy streaming the input loads; this one is empty).
            nc.scalar.dma_start(out=stats_in_d[:, :], in_=stats_sb)
            if skip_collective:
                # timing-only variant: collectives inside a device-side For_i
                # desync the mesh on the 2nd iteration, so the timing loop
                # substitutes a local DRAM copy (output values are wrong by a
                # constant stats factor; latency profile is comparable).
                nc.scalar.dma_start(out=stats_out_d[:, :], in_=stats_in_d[:, :])
            else:
                nc.gpsimd.collective_compute(
                    "AllReduce",
                    ALU.add,
                    replica_groups=[[0, 1, 2, 3], [4, 5, 6, 7]],
                    ins=[stats_in_d[:, :]],
                    outs=[stats_out_d[:, :]],
                )
            g_sb = spool.tile([8, 8], F32, tag="gout")
            nc.scalar.dma_start(out=g_sb, in_=stats_out_d[:, :])

            # Per channel-tile affine coefficients:
            #   A = gamma * rstd,  B = beta - mean*rstd*gamma
            # rstd = exp(-0.5 * ln(var + eps)); the Exp here pays the one
            # on-chain table load, after which the 64 softmax exps are warm.
            cA, cB = [], []
            for t in range(CT):
                m1 = g_sb[:, 2 * t:2 * t + 1]
                m2 = g_sb[:, 2 * t + 1:2 * t + 2]
                var8 = spool.tile([8, 1], F32, tag=f"var{t}")
                nc.vector.tensor_mul(var8, m1, m1)
                nc.vector.tensor_sub(var8, m2, var8)
                nc.scalar.activation(var8, var8, AF.Ln, bias=eps_sb[0:8, :])
                nc.scalar.activation(var8, var8, AF.Exp, scale=-0.5)  # rstd
                ab8 = spool.tile([8, 2], F32, tag=f"ab{t}")
                nc.vector.tensor_copy(out=ab8[:, 0:1], in_=var8)
                nc.vector.tensor_mul(ab8[:, 1:2], m1, var8)  # mean*rstd
                ps_ab = pst.tile([128, 1024], F32, tag="st")
                nc.tensor.matmul(
                    ps_ab[:, 0:2], lhsT=sel_bwd, rhs=ab8, start=True, stop=True
                )
                a_t = spool.tile([128, 1], F32, tag=f"cA{t}")
                b_t = spool.tile([128, 1], F32, tag=f"cB{t}")
                nc.vector.tensor_mul(a_t, ps_ab[:, 0:1], gamma_sb[:, t:t + 1])
                nc.vector.tensor_mul(b_t, ps_ab[:, 1:2], gamma_sb[:, t:t + 1])
                nc.vector.tensor_sub(b_t, beta_sb[:, t:t + 1], b_t)
                cA.append(a_t)
                cB.append(b_t)

            # q0 = Wq^T B per head-pair tile (before Wq is scaled in place).
            q0_sb = []
            for j in range(PAIRS):
                pq0 = pst.tile([128, 1024], F32, tag="st")
                for t in range(CT):
                    nc.tensor.matmul(
                        pq0[:, 0:1],
                        lhsT=wq_sb[t][:, j * 128:(j + 1) * 128].bitcast(F32),
                        rhs=cB[t],
                        start=(t == 0), stop=(t == CT - 1),
                    )
                q0 = spool.tile([128, 1], F32, tag=f"q0{j}")
                nc.vector.tensor_copy(out=q0, in_=pq0[:, 0:1])
                q0_sb.append(q0)
            # Fold A into Wq rows in place (q0 above already consumed raw
            # Wq). Pair 0's slices go on the scalar engine (idle during
            # startup; Copy is in the loaded table set) so qT_0 unblocks
            # ~1us sooner; the remaining pairs' slices go on the also-idle
            # vector engine so they never queue behind the exp stream.
            for t in range(CT):
            

                nc.scalar.activation(
                    wq_sb[t][:, 0:128], wq_sb[t][:, 0:128], AF.Copy,
                    scale=cA[t],
                )
            for t in range(CT):
                nc.vector.tensor_scalar_mul(
                    out=wq_sb[t][:, 128:INNER], in0=wq_sb[t][:, 128:INNER],
                    scalar1=cA[t],
                )

            ot_sb = [
                otpool.tile([128, NL], BF16, tag=f"ot{j}", name=f"ot{j}")
                for j in range(PAIRS)
            ]
            v_sb = []

            def emit_kq(j):
                # K^T_j = (context @ Wk)^T and q^T_j for one head pair.
                kt_ = ktpool.tile([128, CTX], BF16, tag=f"kT{j}")
                for cc in range(2):
                    pk = pp.tile([128, 512], F32, tag="proj")
                    for kk in range(KT):
                        nc.tensor.matmul(
                            pk,
                            lhsT=(wk_sb[kk][:, j * 128:(j + 1) * 128]),
                            rhs=(ctx_sb[kk][:, cc * 512:(cc + 1) * 512]),
                            start=(kk == 0), stop=(kk == KT - 1),
                        )
                    nc.vector.tensor_copy(
                        out=kt_[:, cc * 512:(cc + 1) * 512], in_=pk
                    )
                qt_ = qpool.tile([128, NL], BF16, tag=f"qT{j}")
                for ncc in range(2):
                    pq = pp.tile([128, 512], F32, tag="proj")
                    for t in range(CT):
                        nc.tensor.matmul(
                            pq,
                            lhsT=(wq_sb[t][:, j * 128:(j + 1) * 128]),
                            rhs=(x_sb[t][:, ncc * 512:(ncc + 1) * 512]),
                            start=(t == 0), stop=(t == CT - 1),
                        )
                    nc.vector.tensor_scalar_add(
                        out=qt_[:, ncc * 512:(ncc + 1) * 512],
                        in0=pq,
                        scalar1=q0_sb[j],
                    )
                return kt_, qt_

            def emit_scores(j, kt_, qt_, ncc):
                # Scores for both heads of a pair are issued back-to-back with
                # tile_position (0,0)/(64,0): K=64 matmuls on distinct PE array
                # row-groups run concurrently (~2x). One exp covers both heads.
                cs = slice(ncc * 512, (ncc + 1) * 512)
                e_tiles = []
                for m in range(MT):
                    stp = pst.tile([128, 1024], F32, tag="st")
                    nc.tensor.matmul(
                        stp[:, 0:512],
                        lhsT=kt_[0:HD, m * 128:(m + 1) * 128],
                        rhs=qt_[0:HD, cs],
                        start=True, stop=True,
                    )
                    nc.tensor.matmul(
                        stp[:, 512:1024],
                        lhsT=kt_[HD:128, m * 128:(m + 1) * 128],
                        rhs=qt_[HD:128, cs],
                        start=True, stop=True,
                    )
                    et = epool.tile([128, 1024], BF16, tag="e")
                    nc.scalar.activation(et, stp, AF.Exp, scale=SCALE)
                    e_tiles.append(et)
                return e_tiles

            ones64_sb = consts.tile([1, HD], F32, tag="ones64")
            nc.vector.memset(ones64_sb, 1.0)

            def emit_av(j, ncc, e_tiles, fast_bc=False):
                cs = slice(ncc * 512, (ncc + 1) * 512)
                for half in range(2):
                    h = 2 * j + half
                    rs = slice(half * HD, (half + 1) * HD)
                    es = slice(half * 512, (half + 1) * 512)
                    avp = pav.tile([HD + 1, 512], F32, tag="av")
                    for m in range(MT):
                        nc.tensor.matmul(
                            avp,
                            lhsT=v_sb[m][:, h, :],
                            rhs=e_tiles[m][:, es],
                            start=(m == 0), stop=(m == MT - 1),
                        )
                    rec1 = dpool.tile([1, 512], F32, tag="rec1")
                    nc.vector.reciprocal(out=rec1, in_=avp[HD:HD + 1, :])
                    rec = dpool.tile([HD, 512], F32, tag="rec")
                    if fast_bc:
                        # tail path: broadcast 1/den across partitions with a
                        # K=1 matmul + PSUM evacuation (~2us) instead of the
                        # ~5us DRAM bounce; borrows the idle scores PSUM ring.
                        bc_ps = pst.tile([128, 1024], F32, tag="st")
                        nc.tensor.matmul(
                            bc_ps[0:HD, 0:512], lhsT=ones64_sb, rhs=rec1,
                            start=True, stop=True,
                        )
                        nc.vector.tensor_copy(out=rec, in_=bc_ps[0:HD, 0:512])
                    else:
                        # steady state: DRAM bounce (SBUF/PSUM can't be read
                        # with partition-stride 0, DRAM can); uses the idle
                        # DMA engines instead of the busy vector engine.
                        nc.sync.dma_start(out=den_d[h, ncc, :], in_=rec1)
                        den_row = den_d[h, ncc, :]
                        den_bc_ap = bass.AP(
                            tensor=den_row.tensor,
                            offset=den_row.offset,
                            ap=[[0, HD], [1, 512]],
                        )
                        nc.sync.dma_start(out=rec, in_=den_bc_ap)
                    nc.vector.tensor_mul(ot_sb[j][rs, cs], avp[0:HD, :], rec)

            def emit_v(ms):
                # V = context @ Wv (per ctx row-tile, all heads; +ones col)
                for m in ms:
                    pv = pp.tile([128, 512], F32, tag="proj")
                    for kk in range(KT):
                        nc.tensor.matmul(
                            pv,
                            lhsT=(ctx_sb[kk][:, m * 128:(m + 1) * 128]),
                            rhs=(wv_sb[kk]),
                            start=(kk == 0), stop=(kk == KT - 1),
                        )
                    vt = vpool.tile([128, HEADS, HD + 1], BF16, tag=f"v{m}")
                    nc.vector.tensor_copy(
                        out=vt[:, :, 0:HD],
                        in_=pv.rearrange("p (h d) -> p h d", h=HEADS),
                    )
                    nc.vector.tensor_copy(
                        out=vt[:, :, HD:HD + 1],
                        in_=ones8_sb.rearrange("p (f one) -> p f one", one=1),
                    )
                    v_sb.append(vt)

            # Emission order = tensor-engine priority. Scores (which feed the
            # scalar engine's 64us exp stream) outrank V and AV; V sits after
            # pair 1's scores so those exps keep the scalar engine fed while
            # the PE chews through V's 10us block.
            kq = {0: emit_kq(0)}
            e_saved = {(0, 0): emit_scores(0, kq[0][0], kq[0][1], 0),
                       (0, 1): emit_scores(0, kq[0][0], kq[0][1], 1)}
            # V is the PE's filler work while pair 0's exps drain, but pair
            # 1's projections must beat the exp stream: sandwich them into
            # the V block.
            emit_v(range(0, 5))
            kq[1] = emit_kq(1)
            emit_v(range(5, MT))
            for j in range(1, PAIRS):
                if j > 1:
                    kq[j] = emit_kq(j)
                for ncc in range(2):
                    e_saved[(j, ncc)] = emit_scores(j, kq[j][0], kq[j][1], ncc)
                for ncc in range(2):
                    emit_av(j - 1, ncc, e_saved.pop((j - 1, ncc)))
            for ncc in range(2):
                emit_av(PAIRS - 1, ncc, e_saved.pop((PAIRS - 1, ncc)),
                        fast_bc=True)

            # ---- out = OT^T Wo + bo + residual ----
            for ncc in range(2):
                cs = slice(ncc * 512, (ncc + 1) * 512)
                for t in range(CT):
                    po = pp.tile([128, 512], F32, tag="proj")
                    for jj in range(PAIRS):
                        nc.tensor.matmul(
                            po,
                            lhsT=(wo_sb[jj][:, t * 128:(t + 1) * 128]),
                            rhs=(ot_sb[jj][:, cs]),
                            start=(jj == 0), stop=(jj == PAIRS - 1),
                        )
                    res = opool.tile([128, 512], F32, tag="res")
                    nc.vector.scalar_tensor_tensor(
                        out=res,
                        in0=po,
                        scalar=bo_sb[:, t:t + 1],
                        in1=x_sb[t][:, cs],
                        op0=ALU.add,
                        op1=ALU.add,
                    )
                    nc.sync.dma_start(out=out_d[t, :, cs], in_=res)

    nc.finalize()
    return nc


def _host_prep(x, context, gamma, beta, Wq, Wk, Wv, Wo, bo):
    """Build the 8 per-core input maps (host-side slicing/transposes only)."""
    x2 = np.ascontiguousarray(x, np.float32).reshape(B, C, N)
    ctx = np.ascontiguousarray(context, np.float32)

    sel_fwd = np.zeros((128, 8), np.float32)
    for p in range(128):
        sel_fwd[p, p // 16] = 1.0 / 64.0  # 16 channels x 4 cores
    sel_bwd = np.zeros((8, 128), np.float32)
    for p in range(128):
        sel_bwd[p // 16, p] = 1.0

    def _bf16(a):
        import ml_dtypes
        return np.ascontiguousarray(a).astype(ml_dtypes.bfloat16)

    shared = {
        "wq": np.ascontiguousarray(Wq, np.float32).reshape(CT, 128, INNER),
        "wk": _bf16(np.asarray(Wk, np.float32).reshape(KT, 128, INNER)),
        "wv": _bf16(np.asarray(Wv, np.float32).reshape(KT, 128, INNER)),
        "wo": _bf16(np.asarray(Wo, np.float32).reshape(CT, 128, C)),
        "gamma_t": np.ascontiguousarray(
            np.asarray(gamma, np.float32).reshape(CT, 128).T
        ),
        "beta_t": np.ascontiguousarray(
            np.asarray(beta, np.float32).reshape(CT, 128).T
        ),
        "bo_t": np.ascontiguousarray(np.asarray(bo, np.float32).reshape(CT, 128).T),
        "sel_fwd": sel_fwd,
        "sel_bwd": sel_bwd,
        "stats_in": np.zeros((8, 8), np.float32),
        "stats_out": np.zeros((8, 8), np.float32),
    }

    in_maps = []
    for core in range(8):
        b, qt = core // 4, core % 4
        m = dict(shared)
        m["x_l"] = np.ascontiguousarray(
            x2[b, :, qt * NL:(qt + 1) * NL]
        ).reshape(CT, 128, NL)
        m["ctxT"] = _bf16(ctx[b].T.reshape(KT, 128, CTX))
        in_maps.append(m)
    return in_maps


def _assemble(results):
    out = np.zeros((B, C, N), np.float32)
    for core in range(8):
        b, qt = core // 4, core % 4
        out[b, :, qt * NL:(qt + 1) * NL] = results[core]["out_l"].reshape(C, NL)
    return out.reshape(B, C, 16, 16, 16)


def run(inputs, trace=False):
    global _CACHED_NC
    if _CACHED_NC is None:
        _CACHED_NC = build_nc()
    nc = _CACHED_NC
    in_maps = _host_prep(**inputs)
    # stats_in/stats_out are internal dram tensors, not ExternalInputs
    for m in in_maps:
        m.pop("stats_in")
        m.pop("stats_out")
    bkr = bass_utils.run_bass_kernel_spmd(
        nc, in_maps, list(range(8)), trace=trace
    )
    return _assemble(bkr.results), bkr


def kernel(**inputs):
    out, _ = run(inputs)
    return out
